# revision 1
# baseline (speedup 1.0000x reference)
"""Trainium2 Bass kernel for nn_MultiHeadSelfTokenAttention.

Reference computation (per (b, s) slice, X = hidden[b, s] in [T=128, H=768]):
    q      = X @ Wq + bq                       [T, 12]     (per-token per-head logit)
    scores = q + mask[:, None] * (-10000)
    alpha  = softmax(scores, axis=T)           [T, 12]
    v      = (X @ Wv + bv).reshape(T, 12, 64)
    res    = einsum('th,thd->hd', alpha, v)    [12, 64] -> [768]
    out    = LN(res @ Wo + bo) * gamma + beta  [768]

Key algebraic restructure (makes the kernel memory-bound instead of
compute-bound): the pooled value P = sum_t alpha * V is computed as
    Y[head, h] = sum_t alpha[t, head] * X[t, h]
    P[head, :] = Y[head, :] @ Wv[:, head*64:(head+1)*64] + bv_head
so V ([T, 768] per slice) is never materialized; the X@Wv matmul
(151 MFLOP/slice) collapses to ~2.4 MFLOP/slice.

Sharding: data-parallel across batch; core b handles hidden_states[b]
(32 sents).  Weights replicated.  No collectives.
"""

import os
import sys
from contextlib import ExitStack

import numpy as np

for _p in ("/opt/trn_rl_repo", "/root/.axon_site/_ro/trn_rl_repo"):
    if os.path.isdir(_p) and _p not in sys.path:
        sys.path.insert(0, _p)

import concourse.bacc as bacc
import concourse.bass as bass
import concourse.tile as tile
from concourse import mybir
from concourse.bass_utils import run_bass_kernel_spmd

F32 = mybir.dt.float32
F32R = mybir.dt.float32r
AF = mybir.ActivationFunctionType
ALU = mybir.AluOpType

HIDDEN = 768
HEADS = 12
B, S, T = 8, 32, 128
HC = HIDDEN // 128  # 6 chunks of the hidden dim
LN_EPS = 1e-5
MASK_NEG = -10000.0
N_CORES = 8
BS = int(os.environ.get("KBS", "4"))  # sents per block
NBLK = S // BS
HBS = 4  # ett/yt granularity (slices)

# dtype knob: float32r tiles feed 1-pass (4x faster, ~1.5e-4 rel) PE matmuls;
# float32 gives the exact 2-pass path.
MMD = F32R if os.environ.get("KMM", "f32r") == "f32r" else F32


def build_kernel():
    nc = bacc.Bacc(trn_type="TRN2", target_bir_lowering=False, debug=False)

    hs = nc.dram_tensor("hs", [S, T, HIDDEN], MMD, kind="ExternalInput").ap()
    mask = nc.dram_tensor("mask", [S, T], MMD, kind="ExternalInput").ap()
    wq = nc.dram_tensor("wq", [HIDDEN, HEADS], MMD, kind="ExternalInput").ap()
    bq = nc.dram_tensor("bq", [HEADS], MMD, kind="ExternalInput").ap()
    wv = nc.dram_tensor("wv", [HIDDEN, HIDDEN], MMD, kind="ExternalInput").ap()
    bv = nc.dram_tensor("bv", [HIDDEN], F32, kind="ExternalInput").ap()
    wo = nc.dram_tensor("wo", [HIDDEN, HIDDEN], MMD, kind="ExternalInput").ap()
    bo = nc.dram_tensor("bo", [HIDDEN], F32, kind="ExternalInput").ap()
    gamma = nc.dram_tensor("gamma", [HIDDEN], F32, kind="ExternalInput").ap()
    beta = nc.dram_tensor("beta", [HIDDEN], F32, kind="ExternalInput").ap()
    ident = nc.dram_tensor("ident", [128, 128], MMD, kind="ExternalInput").ap()
    out = nc.dram_tensor("out", [S, HIDDEN], F32, kind="ExternalOutput").ap()

    with tile.TileContext(nc) as tc:
        kernel_body(tc, out, hs, mask, wq, bq, wv, bv, wo, bo, gamma, beta, ident)
    nc.compile()
    return nc


def kernel_body(tc, out, hs, mask, wq, bq, wv, bv, wo, bo, gamma, beta, ident):
    nc = tc.nc
    with ExitStack() as ctx:
        consts = ctx.enter_context(tc.tile_pool(name="consts", bufs=1))
        xp = ctx.enter_context(tc.tile_pool(name="x", bufs=3))
        xtp = ctx.enter_context(tc.tile_pool(name="xt", bufs=2))
        smallp = ctx.enter_context(tc.tile_pool(name="small", bufs=2))
        psctx = ExitStack()
        ps_xt = psctx.enter_context(tc.tile_pool(name="ps_xt", bufs=2, space="PSUM"))
        ps_qt = psctx.enter_context(tc.tile_pool(name="ps_qt", bufs=1, space="PSUM"))
        ps_et = psctx.enter_context(tc.tile_pool(name="ps_et", bufs=1, space="PSUM"))
        ps_yt = psctx.enter_context(tc.tile_pool(name="ps_yt", bufs=1, space="PSUM"))

        # ------------- constants / weights (scalar DMA ring; the sync ring is
        # reserved for X blocks so block 0's load starts immediately) --------
        ident_sb = consts.tile([128, 128], MMD, tag="ident")
        nc.scalar.dma_start(ident_sb[:], ident[:])

        # wq_sb[p, c*12+n] = Wq[c*128+p, n]
        wq_sb = consts.tile([128, HC * HEADS], MMD, tag="wq")
        nc.scalar.dma_start(wq_sb[:], wq.rearrange("(c p) n -> p c n", p=128))

        # extras matmul operands: scores^T += [NEG; bq]^T-style rank-2 update
        extras_w = consts.tile([2, HEADS], MMD, tag="exw")
        nc.vector.memset(extras_w.bitcast(F32)[0:1, :], MASK_NEG)
        nc.scalar.dma_start(extras_w[1:2, :], bq[None, :])
        extras_rhs = consts.tile([2, S * T], MMD, tag="exr")
        nc.vector.memset(extras_rhs.bitcast(F32)[:], 1.0)  # row 1 stays all-ones
        nc.scalar.dma_start(
            extras_rhs[0:1, :], mask.rearrange("s t -> (s t)")[None, :]
        )

        ones_col = consts.tile([1, S], F32, tag="ones")
        nc.vector.memset(ones_col[:], 1.0)
        bo_row = consts.tile([1, HIDDEN], F32, tag="bo")
        nc.scalar.dma_start(bo_row[:], bo[None, :])
        g_row = consts.tile([1, HIDDEN], F32, tag="grow")
        nc.scalar.dma_start(g_row[:], gamma[None, :])
        b_row = consts.tile([1, HIDDEN], F32, tag="brow")
        nc.scalar.dma_start(b_row[:], beta[None, :])

        # bv_sb[p, c] = bv[c*128+p]
        bv_sb = consts.tile([128, HC], F32, tag="bv")
        nc.scalar.dma_start(bv_sb[:], bv.rearrange("(c p) -> p c", p=128))

        # big weight loads issued last on the scalar ring (needed only at G/C)
        # wv_sb[p, c*768+d] = Wv[c*128+p, d]
        wv_sb = consts.tile([128, HC * HIDDEN], MMD, tag="wv")
        nc.scalar.dma_start(wv_sb[:], wv.rearrange("(c p) n -> p c n", p=128))
        wo_sb = consts.tile([128, HC * HIDDEN], MMD, tag="wo")
        nc.scalar.dma_start(wo_sb[:], wo.rearrange("(c p) n -> p c n", p=128))

        # Y^T accumulator in SBUF: per h-chunk [128, S*HEADS], col = s*12 + head
        yt_sb = [
            consts.tile([128, S * HEADS], MMD, tag=f"yt{c}", name=f"yt{c}")
            for c in range(HC)
        ]

        # ---------------- main loop over sent blocks ------------------------
        # Explicit software pipeline: block b's X^T/q^T section is emitted
        # before block b-1's Y^T section so the PE always has independent
        # work during cross-engine softmax round-trips.
        stash = {}

        def stage_a(blk):
            s0 = blk * BS
            x_blk = xp.tile([128, BS * HIDDEN], MMD, tag="xblk", name="x_blk")
            if blk == 0:
                for sp in range(BS):
                    nc.sync.dma_start(
                        x_blk[:, sp * HIDDEN : (sp + 1) * HIDDEN],
                        hs[s0 + sp].rearrange("t h -> t h"),
                    )
            else:
                nc.sync.dma_start(
                    x_blk[:], hs[s0 : s0 + BS].rearrange("s t h -> t s h")
                )

            # X^T block in SBUF: col = s'*768 + hc*128 + j
            xt_blk = xtp.tile([128, BS * HIDDEN], MMD, tag="xtblk", name="xt_blk")
            for sp in range(BS):
                xt_ps = ps_xt.tile([128, HIDDEN], MMD, tag="xtps", name="xt_ps")
                for c in range(HC):
                    nc.tensor.transpose(
                        xt_ps[:, c * 128 : (c + 1) * 128],
                        x_blk[
                            :, sp * HIDDEN + c * 128 : sp * HIDDEN + (c + 1) * 128
                        ],
                        ident_sb[:],
                    )
                nc.vector.tensor_copy(
                    xt_blk[:, sp * HIDDEN : sp * HIDDEN + 512], xt_ps[:, 0:512]
                )
                nc.scalar.copy(
                    xt_blk[:, sp * HIDDEN + 512 : (sp + 1) * HIDDEN],
                    xt_ps[:, 512:768],
                )
            return x_blk, xt_blk

        def stage_q(blk, xt_blk):
            s0 = blk * BS
            qt_ps = ps_qt.tile([HEADS, BS * T], F32, tag="qt", name="qt_ps")
            xt_r = xt_blk.rearrange("p (s c j) -> p c s j", s=BS, j=128)
            spw = 512 // T
            nh = BS // spw
            for c in range(HC):
                for h in range(nh):
                    nc.tensor.matmul(
                        qt_ps[:, h * 512 : (h + 1) * 512],
                        wq_sb[:, c * HEADS : (c + 1) * HEADS],
                        xt_r[:, c, h * spw : (h + 1) * spw],
                        start=(c == 0),
                        stop=False,
                    )
            for h in range(nh):
                nc.tensor.matmul(
                    qt_ps[:, h * 512 : (h + 1) * 512],
                    extras_w[:],
                    extras_rhs[:, s0 * T + h * 512 : s0 * T + (h + 1) * 512],
                    start=False,
                    stop=True,
                )

            # softmax pieces (no max-subtraction: unmasked logits are O(5);
            # masked logits are ~-1e4 and exp underflows to exactly 0)
            et_sb = smallp.tile([HEADS, BS * T], F32, tag="et", name="et_sb")
            zsum = smallp.tile([HEADS, BS], F32, tag="zsum", name="zsum")
            for sp in range(BS):
                nc.scalar.activation(
                    et_sb[:, sp * T : (sp + 1) * T],
                    qt_ps[:, sp * T : (sp + 1) * T],
                    AF.Exp,
                    accum_out=zsum[:, sp : sp + 1],
                )
            zinv = smallp.tile([HEADS, BS], F32, tag="zinv", name="zinv")
            nc.vector.reciprocal(zinv[:], zsum[:])
            return et_sb, zinv

        def stage_b(blk, x_blk, et_sb, zinv):
            s0 = blk * BS
            # normalize: alpha^T = e^T / Z, then transpose to [t, head] layout
            at_sb = smallp.tile([HEADS, BS * T], MMD, tag="at", name="at_sb")
            for sp in range(BS):
                nc.vector.tensor_scalar_mul(
                    at_sb[:, sp * T : (sp + 1) * T],
                    et_sb[:, sp * T : (sp + 1) * T],
                    zinv[:, sp : sp + 1],
                )
            for half in range(BS // HBS):
                ett_ps = ps_et.tile(
                    [128, HBS * HEADS], MMD, tag="ett", name="ett_ps"
                )
                for hp in range(HBS):
                    sp = half * HBS + hp
                    nc.tensor.transpose(
                        ett_ps[:, hp * HEADS : (hp + 1) * HEADS],
                        at_sb[:, sp * T : (sp + 1) * T],
                        ident_sb[0:HEADS, 0:HEADS],
                    )
                e_sb = smallp.tile([128, HBS * HEADS], MMD, tag="e", name="e_sb")
                nc.vector.tensor_copy(e_sb[:], ett_ps[:])

                yt_ps = ps_yt.tile(
                    [128, HC * HBS * HEADS], F32, tag="ytps", name="yt_ps"
                )
                for hp in range(HBS):
                    sp = half * HBS + hp
                    for c in range(HC):
                        nc.tensor.matmul(
                            yt_ps[
                                :,
                                c * HBS * HEADS
                                + hp * HEADS : c * HBS * HEADS
                                + (hp + 1) * HEADS,
                            ],
                            x_blk[
                                :,
                                sp * HIDDEN + c * 128 : sp * HIDDEN + (c + 1) * 128,
                            ],
                            e_sb[:, hp * HEADS : (hp + 1) * HEADS],
                        )
                for c in range(HC):
                    eng = nc.vector.tensor_copy if c % 2 == 0 else nc.scalar.copy
                    eng(
                        yt_sb[c][
                            :,
                            (s0 + half * HBS)
                            * HEADS : (s0 + half * HBS + HBS)
                            * HEADS,
                        ],
                        yt_ps[:, c * HBS * HEADS : (c + 1) * HBS * HEADS],
                    )

        for blk in range(NBLK):
            a = stage_a(blk)
            if blk - 1 in stash:
                stage_b(blk - 1, *stash.pop(blk - 1))
            et_sb, zinv = stage_q(blk, a[1])
            stash[blk] = (a[0], et_sb, zinv)
        stage_b(NBLK - 1, *stash.pop(NBLK - 1))

        psctx.close()  # free the main-loop PSUM banks before stage G

        # ---------------- pooled projection through Wv (G-route) ------------
        # G^T[d, (s,head)] = sum_h Wv[h, d] * Y^T[h, (s,head)]; per-head
        # diagonal blocks extracted:  P^T[d, s] = G^T[d, s*12 + head(d)] + bv[d]
        with (
            tc.tile_pool(name="ps_g", bufs=2, space="PSUM") as ps_g,
            tc.tile_pool(name="ps_o", bufs=1, space="PSUM") as ps_o,
            tc.tile_pool(name="fin", bufs=1) as fin,
        ):
            # gamma/beta replicated across the 32 sent-partitions via K=1 matmuls
            gamma_rep = fin.tile([S, HIDDEN], F32, tag="grep", name="gamma_rep")
            beta_rep = fin.tile([S, HIDDEN], F32, tag="brep", name="beta_rep")
            for row, rep in ((g_row, gamma_rep), (b_row, beta_rep)):
                gb1 = ps_g.tile([S, 512], F32, tag="g", name="gb1")
                gb2 = ps_g.tile([S, 256], F32, tag="g", name="gb2")
                nc.tensor.matmul(gb1[:], ones_col[:], row[:, 0:512])
                nc.tensor.matmul(gb2[:], ones_col[:], row[:, 512:768])
                nc.vector.tensor_copy(rep[:, 0:512], gb1[:])
                nc.scalar.copy(rep[:, 512:768], gb2[:])

            pt_sb = fin.tile([128, HC * S], MMD, tag="pt", name="pt_sb")
            for dc in range(HC):
                g_ps = ps_g.tile([128, S * HEADS], F32, tag="g", name="g_ps")
                for c in range(HC):
                    nc.tensor.matmul(
                        g_ps[:],
                        wv_sb[
                            :, c * HIDDEN + dc * 128 : c * HIDDEN + (dc + 1) * 128
                        ],
                        yt_sb[c][:],
                        start=(c == 0),
                        stop=(c == HC - 1),
                    )
                g_r = g_ps.rearrange("p (s n) -> p s n", n=HEADS)
                for half in range(2):
                    head = 2 * dc + half
                    nc.vector.tensor_scalar_add(
                        pt_sb[half * 64 : half * 64 + 64, dc * S : (dc + 1) * S],
                        g_r[half * 64 : half * 64 + 64, :, head],
                        bv_sb[half * 64 : half * 64 + 64, dc : dc + 1],
                    )

            # out = P @ Wo + bo   -> [32, 768]
            o1 = ps_o.tile([S, 512], F32, tag="o1", name="o1")
            o2 = ps_o.tile([S, 256], F32, tag="o2", name="o2")
            for dc in range(HC):
                nc.tensor.matmul(
                    o1[:],
                    pt_sb[:, dc * S : (dc + 1) * S],
                    wo_sb[:, dc * HIDDEN : dc * HIDDEN + 512],
                    start=(dc == 0),
                    stop=False,
                )
                nc.tensor.matmul(
                    o2[:],
                    pt_sb[:, dc * S : (dc + 1) * S],
                    wo_sb[:, dc * HIDDEN + 512 : (dc + 1) * HIDDEN],
                    start=(dc == 0),
                    stop=False,
                )
            nc.tensor.matmul(
                o1[:], ones_col[:], bo_row[:, 0:512], start=False, stop=True
            )
            nc.tensor.matmul(
                o2[:], ones_col[:], bo_row[:, 512:768], start=False, stop=True
            )

            # ---------------- layernorm ------------------------------------
            res_sb = fin.tile([S, HIDDEN], F32, tag="res", name="res_sb")
            mu_parts = fin.tile([S, 2], F32, tag="mup", name="mu_parts")
            nc.scalar.activation(
                res_sb[:, 0:512], o1[:], AF.Copy, accum_out=mu_parts[:, 0:1]
            )
            nc.scalar.activation(
                res_sb[:, 512:768], o2[:], AF.Copy, accum_out=mu_parts[:, 1:2]
            )
            mu = fin.tile([S, 1], F32, tag="mu", name="mu")
            nc.vector.tensor_reduce(
                mu[:], mu_parts[:], axis=mybir.AxisListType.X, op=ALU.add
            )
            muv = fin.tile([S, 1], F32, tag="muv", name="muv")
            nc.vector.tensor_scalar_mul(muv[:], mu[:], 1.0 / HIDDEN)
            xc = fin.tile([S, HIDDEN], F32, tag="xc", name="xc")
            nc.vector.tensor_scalar_sub(xc[:], res_sb[:], muv[:])
            sq = fin.tile([S, HIDDEN], F32, tag="sq", name="sq")
            varsum = fin.tile([S, 1], F32, tag="vs", name="varsum")
            nc.scalar.activation(sq[:], xc[:], AF.Square, accum_out=varsum[:])
            vareps = fin.tile([S, 1], F32, tag="ve", name="vareps")
            nc.vector.tensor_scalar(
                vareps[:], varsum[:], 1.0 / HIDDEN, LN_EPS, op0=ALU.mult, op1=ALU.add
            )
            sd = fin.tile([S, 1], F32, tag="sd", name="sd")
            nc.scalar.activation(sd[:], vareps[:], AF.Sqrt)
            rstd = fin.tile([S, 1], F32, tag="rstd", name="rstd")
            nc.vector.reciprocal(rstd[:], sd[:])
            t1 = fin.tile([S, HIDDEN], F32, tag="t1", name="t1")
            nc.vector.scalar_tensor_tensor(
                t1[:], xc[:], rstd[:], gamma_rep[:], op0=ALU.mult, op1=ALU.mult
            )
            out_sb = fin.tile([S, HIDDEN], F32, tag="osb", name="out_sb")
            nc.vector.tensor_add(out_sb[:], t1[:], beta_rep[:])
            nc.sync.dma_start(out[:], out_sb[:])


_NC_CACHE = {}


def kernel(hidden_states, mask, Wq, bq, Wv, bv, Wo, bo, gamma, beta):
    if "nc" not in _NC_CACHE:
        _NC_CACHE["nc"] = build_kernel()
    nc = _NC_CACHE["nc"]
    ident = np.eye(128, dtype=np.float32)
    f32 = np.float32

    def cc(a):
        return np.ascontiguousarray(a, dtype=f32)

    in_maps = [
        {
            "hs": cc(hidden_states[b]),
            "mask": cc(mask[b]),
            "wq": cc(Wq),
            "bq": cc(bq),
            "wv": cc(Wv),
            "bv": cc(bv),
            "wo": cc(Wo),
            "bo": cc(bo),
            "gamma": cc(gamma),
            "beta": cc(beta),
            "ident": ident,
        }
        for b in range(N_CORES)
    ]
    res = run_bass_kernel_spmd(nc, in_maps, core_ids=list(range(N_CORES)))
    _NC_CACHE["last_results"] = res
    globals()["_LAST_RESULTS"] = res
    return np.stack([res.results[i]["out"] for i in range(N_CORES)], axis=0)



# revision 7
# speedup vs baseline: 1.0002x; 1.0002x over previous
"""Trainium2 Bass kernel for nn_MultiHeadSelfTokenAttention.

Reference computation (per (b, s) slice, X = hidden[b, s] in [T=128, H=768]):
    q      = X @ Wq + bq                       [T, 12]     (per-token per-head logit)
    scores = q + mask[:, None] * (-10000)
    alpha  = softmax(scores, axis=T)           [T, 12]
    v      = (X @ Wv + bv).reshape(T, 12, 64)
    res    = einsum('th,thd->hd', alpha, v)    [12, 64] -> [768]
    out    = LN(res @ Wo + bo) * gamma + beta  [768]

Algebraic restructure: the pooled value P = sum_t alpha * V is computed as
    Y[head, h] = sum_t alpha[t, head] * X[t, h]
    P[head, :] = Y[head, :] @ Wv[:, head*64:(head+1)*64] + bv_head
so V ([T, 768] per slice) is never materialized.

bf16 datapath (halves HBM traffic, 1-cycle/row PE streaming, FWL weight
loads).  All transposes (X -> X^T for the logit pass, alpha^T -> alpha) run
on the DMA XBAR transpose unit instead of the PE.  Softmax and layernorm
statistics stay in f32.

Sharding: data-parallel across batch; core b handles hidden_states[b]
(32 sents).  Weights replicated.  No collectives.
"""

import os
import sys
from contextlib import ExitStack

import numpy as np
import ml_dtypes

for _p in ("/opt/trn_rl_repo", "/root/.axon_site/_ro/trn_rl_repo"):
    if os.path.isdir(_p) and _p not in sys.path:
        sys.path.insert(0, _p)

import concourse.bacc as bacc
import concourse.bass as bass
import concourse.tile as tile
from concourse import mybir
from concourse.bass_utils import run_bass_kernel_spmd

F32 = mybir.dt.float32
BF16 = mybir.dt.bfloat16
AF = mybir.ActivationFunctionType
ALU = mybir.AluOpType

HIDDEN = 768
HEADS = 12
B, S, T = 8, 32, 128
HC = HIDDEN // 128  # 6 chunks of the hidden dim
LN_EPS = 1e-5
MASK_NEG = -10000.0
N_CORES = 8
BS = 8            # sents per block
NBLK = S // BS    # 4 blocks
# XBAR transpose source: "sbuf" reads the already-loaded x_blk (no extra HBM
# traffic); "dram" re-reads hidden_states from HBM.
XSRC = os.environ.get("KXSRC", "sbuf")


def build_kernel():
    nc = bacc.Bacc(trn_type="TRN2", target_bir_lowering=False, debug=False)

    hs = nc.dram_tensor("hs", [S, T, HIDDEN], BF16, kind="ExternalInput").ap()
    mask = nc.dram_tensor("mask", [S * T], BF16, kind="ExternalInput").ap()
    # host-side pre-rearranged weights
    wq = nc.dram_tensor("wq", [128, HC * HEADS], BF16, kind="ExternalInput").ap()
    bq = nc.dram_tensor("bq", [HEADS], BF16, kind="ExternalInput").ap()
    wv = nc.dram_tensor("wv", [128, HC * HIDDEN], BF16, kind="ExternalInput").ap()
    bv = nc.dram_tensor("bv", [128, HC], F32, kind="ExternalInput").ap()
    wo = nc.dram_tensor("wo", [128, HC * HIDDEN], BF16, kind="ExternalInput").ap()
    bo = nc.dram_tensor("bo", [HIDDEN], BF16, kind="ExternalInput").ap()
    grep = nc.dram_tensor("grep", [S, HIDDEN], F32, kind="ExternalInput").ap()
    brep = nc.dram_tensor("brep", [S, HIDDEN], F32, kind="ExternalInput").ap()
    out = nc.dram_tensor("out", [S, HIDDEN], F32, kind="ExternalOutput").ap()

    with tile.TileContext(nc) as tc:
        kernel_body(tc, out, hs, mask, wq, bq, wv, bv, wo, bo, grep, brep)
    nc.compile()
    return nc


def kernel_body(tc, out, hs, mask, wq, bq, wv, bv, wo, bo, grep, brep):
    nc = tc.nc
    with ExitStack() as ctx:
        consts = ctx.enter_context(tc.tile_pool(name="consts", bufs=1))
        xp = ctx.enter_context(tc.tile_pool(name="x", bufs=3))
        xtp = ctx.enter_context(tc.tile_pool(name="xt", bufs=2))
        smallp = ctx.enter_context(tc.tile_pool(name="small", bufs=2))
        fin = ctx.enter_context(tc.tile_pool(name="fin", bufs=1))
        ps_yt = ctx.enter_context(tc.tile_pool(name="ps_yt", bufs=2, space="PSUM"))
        ps_aq = ExitStack()
        ps_qt = ps_aq.enter_context(tc.tile_pool(name="ps_qt", bufs=2, space="PSUM"))

        # ------------- small constants (scalar ring; X blocks go on the sync
        # ring, the big weights follow the X blocks on the sync ring) --------
        wq_sb = consts.tile([128, HC * HEADS], BF16, tag="wq")
        nc.scalar.dma_start(wq_sb[:], wq[:])

        # extras matmul operands: scores^T += [NEG; bq]^T-style rank-2 update
        extras_w = consts.tile([2, HEADS], BF16, tag="exw")
        nc.vector.memset(extras_w[0:1, :], MASK_NEG)
        nc.scalar.dma_start(extras_w[1:2, :], bq[None, :])
        extras_rhs = consts.tile([2, S * T], BF16, tag="exr")
        nc.vector.memset(extras_rhs[:], 1.0)  # row 1 stays all-ones
        nc.scalar.dma_start(extras_rhs[0:1, :], mask[None, :])

        ones_col = consts.tile([1, S], BF16, tag="ones")
        nc.vector.memset(ones_col[:], 1.0)
        bo_row = consts.tile([1, HIDDEN], BF16, tag="bo")
        nc.scalar.dma_start(bo_row[:], bo[None, :])
        bv_sb = consts.tile([128, HC], F32, tag="bv")
        nc.scalar.dma_start(bv_sb[:], bv[:])

        # big weights: declared here, DMA'd late on the sync ring (see sched)
        wv_sb = consts.tile([128, HC * HIDDEN], BF16, tag="wv")
        wo_sb = consts.tile([128, HC * HIDDEN], BF16, tag="wo")
        gamma_rep = consts.tile([S, HIDDEN], F32, tag="grep")
        beta_rep = consts.tile([S, HIDDEN], F32, tag="brep")

        # Y^T accumulator in SBUF: yt_all[p, c*S*12 + s*12 + n]
        yt_all = consts.tile(
            [128, HC * S * HEADS], BF16, tag="yt", name="yt_all"
        )

        # warm the ACT tables (Exp for softmax, Sqrt for LN) during DMA wait
        warm = fin.tile([1, 2], F32, tag="warm", name="warm")
        nc.vector.memset(warm[:], 1.0)
        warm2 = fin.tile([1, 2], F32, tag="warm2", name="warm2")
        nc.scalar.activation(warm2[:, 0:1], warm[:, 0:1], AF.Sqrt)
        nc.scalar.activation(warm2[:, 1:2], warm[:, 1:2], AF.Exp)

        # ---------------- pipeline stages ------------------------------------
        def stage_a(blk):
            """DMA one block of X (sync ring) + XBAR-transpose it (scalar)."""
            s0 = blk * BS
            x_blk = xp.tile([128, BS * HIDDEN], BF16, tag="xblk", name="x_blk")
            if blk == 0:
                for sp in range(BS):
                    nc.sync.dma_start(
                        x_blk[:, sp * HIDDEN : (sp + 1) * HIDDEN], hs[s0 + sp]
                    )
            else:
                nc.sync.dma_start(
                    x_blk[:], hs[s0 : s0 + BS].rearrange("s t h -> t s h")
                )

            # X^T block in SBUF: col = s'*768 + c*128 + t
            xt_blk = xtp.tile([128, BS * HIDDEN], BF16, tag="xtblk", name="xt_blk")
            for sp in range(BS):
                dst = xt_blk[:, sp * HIDDEN : (sp + 1) * HIDDEN].rearrange(
                    "p (c t) -> p c t", t=128
                )
                if XSRC == "sbuf":
                    src = x_blk[:, sp * HIDDEN : (sp + 1) * HIDDEN]
                else:
                    src = hs[s0 + sp]
                nc.scalar.dma_start(dst, src, transpose=True)
            return x_blk, xt_blk

        def stage_q(blk, xt_blk):
            """q^T logits + masked softmax -> padded alpha^T (bf16)."""
            s0 = blk * BS
            qt_ps = ps_qt.tile([HEADS, BS * T], F32, tag="qt", name="qt_ps")
            xt_r = xt_blk.rearrange("p (s c j) -> p c s j", s=BS, j=128)
            spw = 512 // T  # sents per 512-col matmul
            nh = BS // spw
            for h in range(nh):
                for c in range(HC):
                    nc.tensor.matmul(
                        qt_ps[:, h * 512 : (h + 1) * 512],
                        wq_sb[:, c * HEADS : (c + 1) * HEADS],
                        xt_r[:, c, h * spw : (h + 1) * spw],
                        start=(c == 0),
                        stop=False,
                    )
                nc.tensor.matmul(
                    qt_ps[:, h * 512 : (h + 1) * 512],
                    extras_w[:],
                    extras_rhs[:, s0 * T + h * 512 : s0 * T + (h + 1) * 512],
                    start=False,
                    stop=True,
                )

            # softmax pieces (no max-subtraction: unmasked logits are O(5);
            # masked logits are ~-1e4 and exp underflows to exactly 0)
            et_sb = smallp.tile([HEADS, BS * T], F32, tag="et", name="et_sb")
            zsum = smallp.tile([HEADS, BS], F32, tag="zsum", name="zsum")
            for sp in range(BS):
                nc.scalar.activation(
                    et_sb[:, sp * T : (sp + 1) * T],
                    qt_ps[:, sp * T : (sp + 1) * T],
                    AF.Exp,
                    accum_out=zsum[:, sp : sp + 1],
                )
            zinv = smallp.tile([HEADS, BS], F32, tag="zinv", name="zinv")
            nc.vector.reciprocal(zinv[:], zsum[:])
            # alpha^T padded to 16 partitions for the XBAR (rows 12:16 zero)
            at_sb = smallp.tile([16, BS * T], BF16, tag="at", name="at_sb")
            nc.vector.memset(at_sb[:], 0.0)
            for sp in range(BS):
                nc.vector.tensor_scalar_mul(
                    at_sb[0:HEADS, sp * T : (sp + 1) * T],
                    et_sb[:, sp * T : (sp + 1) * T],
                    zinv[:, sp : sp + 1],
                )
            return at_sb

        def stage_ty(blk, x_blk, at_sb):
            """XBAR alpha^T -> alpha, then Y^T = X^T @ alpha on the PE."""
            e_sb = smallp.tile([128, BS * 16], BF16, tag="e", name="e_sb")
            nc.scalar.dma_start(
                e_sb.rearrange("p (s n) -> p s n", n=16), at_sb[:], transpose=True
            )
            for hb in range(2):
                yt_ps = ps_yt.tile(
                    [128, HC * 4 * HEADS], F32, tag="ytps", name="yt_ps"
                )
                for hp in range(4):
                    sp = hb * 4 + hp
                    for c in range(HC):
                        nc.tensor.matmul(
                            yt_ps[
                                :,
                                c * 4 * HEADS
                                + hp * HEADS : c * 4 * HEADS
                                + (hp + 1) * HEADS,
                            ],
                            x_blk[
                                :,
                                sp * HIDDEN + c * 128 : sp * HIDDEN + (c + 1) * 128,
                            ],
                            e_sb[:, sp * 16 : sp * 16 + HEADS],
                        )
                off = (blk * BS + hb * 4) * HEADS
                dst = yt_all.rearrange("p (c k) -> p c k", c=HC)[
                    :, :, off : off + 4 * HEADS
                ]
                src = yt_ps.rearrange("p (c k) -> p c k", c=HC)
                eng = nc.vector.tensor_copy if hb == 0 else nc.scalar.copy
                eng(dst, src)

        # ---------------- schedule -------------------------------------------
        a0 = stage_a(0)
        at0 = stage_q(0, a0[1])
        a1 = stage_a(1)
        at1 = stage_q(1, a1[1])
        stage_ty(0, a0[0], at0)
        a2 = stage_a(2)
        at2 = stage_q(2, a2[1])
        stage_ty(1, a1[0], at1)
        a3 = stage_a(3)
        at3 = stage_q(3, a3[1])
        # big weights on the sync ring, behind the four X block loads
        nc.sync.dma_start(wv_sb[:], wv[:])
        nc.sync.dma_start(wo_sb[:], wo[:])
        nc.sync.dma_start(gamma_rep[:], grep[:])
        nc.sync.dma_start(beta_rep[:], brep[:])
        stage_ty(2, a2[0], at2)
        stage_ty(3, a3[0], at3)
        ps_aq.close()  # free qt PSUM banks for the endgame pools

        # ---------------- endgame: G-route + out-proj + layernorm ------------
        with (
            tc.tile_pool(name="ps_g", bufs=2, space="PSUM") as ps_g,
            tc.tile_pool(name="ps_o", bufs=1, space="PSUM") as ps_o,
        ):
            pt_sb = fin.tile([128, HC * S], BF16, tag="pt", name="pt_sb")
            for dc in range(HC):
                g_ps = ps_g.tile([128, S * HEADS], F32, tag="g", name="g_ps")
                for c in range(HC):
                    nc.tensor.matmul(
                        g_ps[:],
                        wv_sb[
                            :, c * HIDDEN + dc * 128 : c * HIDDEN + (dc + 1) * 128
                        ],
                        yt_all[:, c * S * HEADS : (c + 1) * S * HEADS],
                        start=(c == 0),
                        stop=(c == HC - 1),
                    )
                g_r = g_ps.rearrange("p (s n) -> p s n", n=HEADS)
                for hh in range(2):
                    head = 2 * dc + hh
                    nc.vector.tensor_scalar_add(
                        pt_sb[hh * 64 : hh * 64 + 64, dc * S : (dc + 1) * S],
                        g_r[hh * 64 : hh * 64 + 64, :, head],
                        bv_sb[hh * 64 : hh * 64 + 64, dc : dc + 1],
                    )

            # out = P @ Wo + bo   -> [32, 768]
            o1 = ps_o.tile([S, 512], F32, tag="o1", name="o1")
            o2 = ps_o.tile([S, 256], F32, tag="o2", name="o2")
            for dc in range(HC):
                nc.tensor.matmul(
                    o1[:],
                    pt_sb[:, dc * S : (dc + 1) * S],
                    wo_sb[:, dc * HIDDEN : dc * HIDDEN + 512],
                    start=(dc == 0),
                    stop=False,
                )
                nc.tensor.matmul(
                    o2[:],
                    pt_sb[:, dc * S : (dc + 1) * S],
                    wo_sb[:, dc * HIDDEN + 512 : (dc + 1) * HIDDEN],
                    start=(dc == 0),
                    stop=False,
                )
            nc.tensor.matmul(
                o1[:], ones_col[:], bo_row[:, 0:512], start=False, stop=True
            )
            nc.tensor.matmul(
                o2[:], ones_col[:], bo_row[:, 512:768], start=False, stop=True
            )

            # ---------------- layernorm ------------------------------------
            res_sb = fin.tile([S, HIDDEN], F32, tag="res", name="res_sb")
            mu_parts = fin.tile([S, 2], F32, tag="mup", name="mu_parts")
            nc.scalar.activation(
                res_sb[:, 0:512], o1[:], AF.Copy, accum_out=mu_parts[:, 0:1]
            )
            nc.scalar.activation(
                res_sb[:, 512:768], o2[:], AF.Copy, accum_out=mu_parts[:, 1:2]
            )
            mu = fin.tile([S, 1], F32, tag="mu", name="mu")
            nc.vector.tensor_reduce(
                mu[:], mu_parts[:], axis=mybir.AxisListType.X, op=ALU.add
            )
            muv = fin.tile([S, 1], F32, tag="muv", name="muv")
            nc.vector.tensor_scalar_mul(muv[:], mu[:], 1.0 / HIDDEN)
            xc = fin.tile([S, HIDDEN], F32, tag="xc", name="xc")
            nc.vector.tensor_scalar_sub(xc[:], res_sb[:], muv[:])
            sq = fin.tile([S, HIDDEN], F32, tag="sq", name="sq")
            varsum = fin.tile([S, 1], F32, tag="vs", name="varsum")
            nc.scalar.activation(sq[:], xc[:], AF.Square, accum_out=varsum[:])
            vareps = fin.tile([S, 1], F32, tag="ve", name="vareps")
            nc.vector.tensor_scalar(
                vareps[:], varsum[:], 1.0 / HIDDEN, LN_EPS, op0=ALU.mult, op1=ALU.add
            )
            sd = fin.tile([S, 1], F32, tag="sd", name="sd")
            nc.scalar.activation(sd[:], vareps[:], AF.Sqrt)
            rstd = fin.tile([S, 1], F32, tag="rstd", name="rstd")
            nc.vector.reciprocal(rstd[:], sd[:])
            t1 = fin.tile([S, HIDDEN], F32, tag="t1", name="t1")
            nc.vector.scalar_tensor_tensor(
                t1[:], xc[:], rstd[:], gamma_rep[:], op0=ALU.mult, op1=ALU.mult
            )
            out_sb = fin.tile([S, HIDDEN], F32, tag="osb", name="out_sb")
            nc.vector.tensor_add(out_sb[:], t1[:], beta_rep[:])
            nc.sync.dma_start(out[:], out_sb[:])


_NC_CACHE = {}


def kernel(hidden_states, mask, Wq, bq, Wv, bv, Wo, bo, gamma, beta):
    if "nc" not in _NC_CACHE:
        _NC_CACHE["nc"] = build_kernel()
    nc = _NC_CACHE["nc"]
    bf = ml_dtypes.bfloat16
    f32 = np.float32

    Wq_r = np.ascontiguousarray(
        np.asarray(Wq, dtype=f32).reshape(HC, 128, HEADS).transpose(1, 0, 2)
        .reshape(128, HC * HEADS).astype(bf)
    )
    Wv_r = np.ascontiguousarray(
        np.asarray(Wv, dtype=f32).reshape(HC, 128, HIDDEN).transpose(1, 0, 2)
        .reshape(128, HC * HIDDEN).astype(bf)
    )
    Wo_r = np.ascontiguousarray(
        np.asarray(Wo, dtype=f32).reshape(HC, 128, HIDDEN).transpose(1, 0, 2)
        .reshape(128, HC * HIDDEN).astype(bf)
    )
    bv_r = np.ascontiguousarray(
        np.asarray(bv, dtype=f32).reshape(HC, 128).T
    )
    bq_b = np.asarray(bq, dtype=f32).astype(bf)
    bo_b = np.asarray(bo, dtype=f32).astype(bf)
    grep = np.ascontiguousarray(np.tile(np.asarray(gamma, dtype=f32), (S, 1)))
    brep = np.ascontiguousarray(np.tile(np.asarray(beta, dtype=f32), (S, 1)))

    in_maps = [
        {
            "hs": np.ascontiguousarray(np.asarray(hidden_states[b], dtype=f32)).astype(bf),
            "mask": np.ascontiguousarray(
                np.asarray(mask[b], dtype=f32).reshape(S * T)
            ).astype(bf),
            "wq": Wq_r,
            "bq": bq_b,
            "wv": Wv_r,
            "bv": bv_r,
            "wo": Wo_r,
            "bo": bo_b,
            "grep": grep,
            "brep": brep,
        }
        for b in range(N_CORES)
    ]
    res = run_bass_kernel_spmd(nc, in_maps, core_ids=list(range(N_CORES)))
    _NC_CACHE["last_results"] = res
    globals()["_LAST_RESULTS"] = res
    return np.stack([res.results[i]["out"] for i in range(N_CORES)], axis=0)


# revision 12
# speedup vs baseline: 1.7486x; 1.7483x over previous
"""Trainium2 Bass kernel for nn_MultiHeadSelfTokenAttention.

Reference computation (per (b, s) slice, X = hidden[b, s] in [T=128, H=768]):
    q      = X @ Wq + bq                       [T, 12]     (per-token per-head logit)
    scores = q + mask[:, None] * (-10000)
    alpha  = softmax(scores, axis=T)           [T, 12]
    v      = (X @ Wv + bv).reshape(T, 12, 64)
    res    = einsum('th,thd->hd', alpha, v)    [12, 64] -> [768]
    out    = LN(res @ Wo + bo) * gamma + beta  [768]

Algebraic restructure: the pooled value P = sum_t alpha * V is computed as
    Y[head, h] = sum_t alpha[t, head] * X[t, h]
    P[head, :] = Y[head, :] @ Wv[:, head*64:(head+1)*64] + bv_head
so V ([T, 768] per slice) is never materialized.

bf16 datapath (halves HBM traffic, 1-cycle/row PE streaming, FWL weight
loads).  X -> X^T for the logit pass runs on the PE (identity transposes);
the small alpha^T -> alpha transposes run on the DMA XBAR transpose unit
(one instruction per 8-sent block).  Softmax and layernorm statistics stay
in f32.

Sharding: data-parallel across batch; core b handles hidden_states[b]
(32 sents).  Weights replicated.  No collectives.
"""

import os
import sys
from contextlib import ExitStack

import numpy as np
import ml_dtypes

for _p in ("/opt/trn_rl_repo", "/root/.axon_site/_ro/trn_rl_repo"):
    if os.path.isdir(_p) and _p not in sys.path:
        sys.path.insert(0, _p)

import concourse.bacc as bacc
import concourse.bass as bass
import concourse.tile as tile
from concourse import mybir
from concourse.bass_utils import run_bass_kernel_spmd

F32 = mybir.dt.float32
BF16 = mybir.dt.bfloat16
AF = mybir.ActivationFunctionType
ALU = mybir.AluOpType

HIDDEN = 768
HEADS = 12
B, S, T = 8, 32, 128
HC = HIDDEN // 128  # 6 chunks of the hidden dim
LN_EPS = 1e-5
MASK_NEG = -10000.0
N_CORES = 8
BS = 8            # sents per block
NBLK = S // BS    # 4 blocks


def build_kernel():
    nc = bacc.Bacc(trn_type="TRN2", target_bir_lowering=False, debug=False)

    hs = nc.dram_tensor("hs", [S, T, HIDDEN], BF16, kind="ExternalInput").ap()
    mask = nc.dram_tensor("mask", [S * T], BF16, kind="ExternalInput").ap()
    # host-side pre-rearranged weights
    wq = nc.dram_tensor("wq", [128, HC * HEADS], BF16, kind="ExternalInput").ap()
    bq = nc.dram_tensor("bq", [HEADS], BF16, kind="ExternalInput").ap()
    wv = nc.dram_tensor("wv", [128, HC * HIDDEN], BF16, kind="ExternalInput").ap()
    bv = nc.dram_tensor("bv", [128, HC], F32, kind="ExternalInput").ap()
    wo = nc.dram_tensor("wo", [128, HC * HIDDEN], BF16, kind="ExternalInput").ap()
    bo = nc.dram_tensor("bo", [HIDDEN], BF16, kind="ExternalInput").ap()
    grep = nc.dram_tensor("grep", [S, HIDDEN], F32, kind="ExternalInput").ap()
    brep = nc.dram_tensor("brep", [S, HIDDEN], F32, kind="ExternalInput").ap()
    ident = nc.dram_tensor("ident", [128, 128], BF16, kind="ExternalInput").ap()
    out = nc.dram_tensor("out", [S, HIDDEN], F32, kind="ExternalOutput").ap()

    with tile.TileContext(nc) as tc:
        kernel_body(tc, out, hs, mask, wq, bq, wv, bv, wo, bo, grep, brep, ident)
    nc.compile()
    return nc


def kernel_body(tc, out, hs, mask, wq, bq, wv, bv, wo, bo, grep, brep, ident):
    nc = tc.nc
    with ExitStack() as ctx:
        consts = ctx.enter_context(tc.tile_pool(name="consts", bufs=1))
        xp = ctx.enter_context(tc.tile_pool(name="x", bufs=3))
        xtp = ctx.enter_context(tc.tile_pool(name="xt", bufs=2))
        smallp = ctx.enter_context(tc.tile_pool(name="small", bufs=2))
        fin = ctx.enter_context(tc.tile_pool(name="fin", bufs=1))
        ps_yt = ctx.enter_context(tc.tile_pool(name="ps_yt", bufs=2, space="PSUM"))
        ps_aq = ExitStack()
        ps_xt = ps_aq.enter_context(tc.tile_pool(name="ps_xt", bufs=2, space="PSUM"))
        ps_qt = ps_aq.enter_context(tc.tile_pool(name="ps_qt", bufs=2, space="PSUM"))

        # ------------- small constants (scalar ring; X blocks go on the sync
        # ring, the big weights follow the X blocks on the sync ring) --------
        wq_sb = consts.tile([128, HC * HEADS], BF16, tag="wq")
        nc.scalar.dma_start(wq_sb[:], wq[:])

        # extras matmul operands: scores^T += [NEG; bq]^T-style rank-2 update
        extras_w = consts.tile([2, HEADS], BF16, tag="exw")
        nc.vector.memset(extras_w[0:1, :], MASK_NEG)
        nc.scalar.dma_start(extras_w[1:2, :], bq[None, :])
        extras_rhs = consts.tile([2, S * T], BF16, tag="exr")
        nc.vector.memset(extras_rhs[:], 1.0)  # row 1 stays all-ones
        nc.scalar.dma_start(extras_rhs[0:1, :], mask[None, :])

        ones_col = consts.tile([1, S], BF16, tag="ones")
        nc.vector.memset(ones_col[:], 1.0)
        bo_row = consts.tile([1, HIDDEN], BF16, tag="bo")
        nc.scalar.dma_start(bo_row[:], bo[None, :])
        bv_sb = consts.tile([128, HC], F32, tag="bv")
        nc.scalar.dma_start(bv_sb[:], bv[:])

        # big weights: declared here, DMA'd late on the sync ring (see sched)
        wv_sb = consts.tile([128, HC * HIDDEN], BF16, tag="wv")
        wo_sb = consts.tile([128, HC * HIDDEN], BF16, tag="wo")
        gamma_rep = consts.tile([S, HIDDEN], F32, tag="grep")
        beta_rep = consts.tile([S, HIDDEN], F32, tag="brep")

        # Y^T accumulator in SBUF: yt_all[p, c*S*12 + s*12 + n]
        yt_all = consts.tile(
            [128, HC * S * HEADS], BF16, tag="yt", name="yt_all"
        )

        # identity for PE transposes
        ident_sb = consts.tile([128, 128], BF16, tag="ident")
        nc.scalar.dma_start(ident_sb[:], ident[:])

        # warm the ACT Exp table during the initial DMA wait
        warm = fin.tile([1, 2], F32, tag="warm", name="warm")
        nc.vector.memset(warm[:], 1.0)
        warm2 = fin.tile([1, 2], F32, tag="warm2", name="warm2")
        nc.scalar.activation(warm2[:, 1:2], warm[:, 1:2], AF.Exp)

        # ---------------- pipeline stages ------------------------------------
        def stage_a(blk):
            """DMA one block of X (sync ring) + XBAR-transpose it (scalar)."""
            s0 = blk * BS
            x_blk = xp.tile([128, BS * HIDDEN], BF16, tag="xblk", name="x_blk")
            if blk == 0:
                for sp in range(BS):
                    nc.sync.dma_start(
                        x_blk[:, sp * HIDDEN : (sp + 1) * HIDDEN], hs[s0 + sp]
                    )
            else:
                nc.sync.dma_start(
                    x_blk[:], hs[s0 : s0 + BS].rearrange("s t h -> t s h")
                )

            # X^T block in SBUF: col = s'*768 + c*128 + t  (PE transposes)
            xt_blk = xtp.tile([128, BS * HIDDEN], BF16, tag="xtblk", name="xt_blk")
            for sp in range(BS):
                xt_ps = ps_xt.tile([128, HIDDEN], BF16, tag="xtps", name="xt_ps")
                for c in range(HC):
                    nc.tensor.transpose(
                        xt_ps[:, c * 128 : (c + 1) * 128],
                        x_blk[
                            :, sp * HIDDEN + c * 128 : sp * HIDDEN + (c + 1) * 128
                        ],
                        ident_sb[:],
                    )
                eng = nc.vector.tensor_copy if sp % 2 == 0 else nc.scalar.copy
                eng(xt_blk[:, sp * HIDDEN : (sp + 1) * HIDDEN], xt_ps[:])
            return x_blk, xt_blk

        def stage_q(blk, xt_blk):
            """q^T logits + masked softmax -> padded alpha^T (bf16)."""
            s0 = blk * BS
            qt_ps = ps_qt.tile([HEADS, BS * T], F32, tag="qt", name="qt_ps")
            xt_r = xt_blk.rearrange("p (s c j) -> p c s j", s=BS, j=128)
            spw = 512 // T  # sents per 512-col matmul
            nh = BS // spw
            for h in range(nh):
                for c in range(HC):
                    nc.tensor.matmul(
                        qt_ps[:, h * 512 : (h + 1) * 512],
                        wq_sb[:, c * HEADS : (c + 1) * HEADS],
                        xt_r[:, c, h * spw : (h + 1) * spw],
                        start=(c == 0),
                        stop=False,
                    )
                nc.tensor.matmul(
                    qt_ps[:, h * 512 : (h + 1) * 512],
                    extras_w[:],
                    extras_rhs[:, s0 * T + h * 512 : s0 * T + (h + 1) * 512],
                    start=False,
                    stop=True,
                )

            # softmax pieces (no max-subtraction: unmasked logits are O(5);
            # masked logits are ~-1e4 and exp underflows to exactly 0)
            et_sb = smallp.tile([HEADS, BS * T], F32, tag="et", name="et_sb")
            zsum = smallp.tile([HEADS, BS], F32, tag="zsum", name="zsum")
            for sp in range(BS):
                nc.scalar.activation(
                    et_sb[:, sp * T : (sp + 1) * T],
                    qt_ps[:, sp * T : (sp + 1) * T],
                    AF.Exp,
                    accum_out=zsum[:, sp : sp + 1],
                )
            zinv = smallp.tile([HEADS, BS], F32, tag="zinv", name="zinv")
            nc.vector.reciprocal(zinv[:], zsum[:])
            # alpha^T padded to 16 partitions for the XBAR (rows 12:16 zero)
            at_sb = smallp.tile([16, BS * T], BF16, tag="at", name="at_sb")
            nc.vector.memset(at_sb[:], 0.0)
            for sp in range(BS):
                nc.vector.tensor_scalar_mul(
                    at_sb[0:HEADS, sp * T : (sp + 1) * T],
                    et_sb[:, sp * T : (sp + 1) * T],
                    zinv[:, sp : sp + 1],
                )
            return at_sb

        def stage_ty(blk, x_blk, at_sb):
            """XBAR alpha^T -> alpha, then Y^T = X^T @ alpha on the PE."""
            e_sb = smallp.tile([128, BS * 16], BF16, tag="e", name="e_sb")
            nc.scalar.dma_start(
                e_sb.rearrange("p (s n) -> p s n", n=16), at_sb[:], transpose=True
            )
            for hb in range(2):
                yt_ps = ps_yt.tile(
                    [128, HC * 4 * HEADS], F32, tag="ytps", name="yt_ps"
                )
                for hp in range(4):
                    sp = hb * 4 + hp
                    for c in range(HC):
                        nc.tensor.matmul(
                            yt_ps[
                                :,
                                c * 4 * HEADS
                                + hp * HEADS : c * 4 * HEADS
                                + (hp + 1) * HEADS,
                            ],
                            x_blk[
                                :,
                                sp * HIDDEN + c * 128 : sp * HIDDEN + (c + 1) * 128,
                            ],
                            e_sb[:, sp * 16 : sp * 16 + HEADS],
                        )
                off = (blk * BS + hb * 4) * HEADS
                dst = yt_all.rearrange("p (c k) -> p c k", c=HC)[
                    :, :, off : off + 4 * HEADS
                ]
                src = yt_ps.rearrange("p (c k) -> p c k", c=HC)
                eng = nc.vector.tensor_copy if hb == 0 else nc.scalar.copy
                eng(dst, src)

        # ---------------- schedule -------------------------------------------
        a0 = stage_a(0)
        at0 = stage_q(0, a0[1])
        a1 = stage_a(1)
        at1 = stage_q(1, a1[1])
        stage_ty(0, a0[0], at0)
        a2 = stage_a(2)
        at2 = stage_q(2, a2[1])
        stage_ty(1, a1[0], at1)
        a3 = stage_a(3)
        at3 = stage_q(3, a3[1])
        # big weights on the sync ring, behind the four X block loads
        nc.sync.dma_start(wv_sb[:], wv[:])
        nc.sync.dma_start(wo_sb[:], wo[:])
        nc.sync.dma_start(gamma_rep[:], grep[:])
        nc.sync.dma_start(beta_rep[:], brep[:])
        stage_ty(2, a2[0], at2)
        stage_ty(3, a3[0], at3)
        ps_aq.close()  # free qt PSUM banks for the endgame pools

        # ---------------- endgame: G-route + out-proj + layernorm ------------
        with (
            tc.tile_pool(name="ps_g", bufs=2, space="PSUM") as ps_g,
            tc.tile_pool(name="ps_o", bufs=1, space="PSUM") as ps_o,
        ):
            pt_sb = fin.tile([128, HC * S], BF16, tag="pt", name="pt_sb")
            for dc in range(HC):
                g_ps = ps_g.tile([128, S * HEADS], F32, tag="g", name="g_ps")
                for c in range(HC):
                    nc.tensor.matmul(
                        g_ps[:],
                        wv_sb[
                            :, c * HIDDEN + dc * 128 : c * HIDDEN + (dc + 1) * 128
                        ],
                        yt_all[:, c * S * HEADS : (c + 1) * S * HEADS],
                        start=(c == 0),
                        stop=(c == HC - 1),
                    )
                g_r = g_ps.rearrange("p (s n) -> p s n", n=HEADS)
                for hh in range(2):
                    head = 2 * dc + hh
                    nc.vector.tensor_scalar_add(
                        pt_sb[hh * 64 : hh * 64 + 64, dc * S : (dc + 1) * S],
                        g_r[hh * 64 : hh * 64 + 64, :, head],
                        bv_sb[hh * 64 : hh * 64 + 64, dc : dc + 1],
                    )

            # out = P @ Wo + bo   -> [32, 768]
            o1 = ps_o.tile([S, 512], F32, tag="o1", name="o1")
            o2 = ps_o.tile([S, 256], F32, tag="o2", name="o2")
            for dc in range(HC):
                nc.tensor.matmul(
                    o1[:],
                    pt_sb[:, dc * S : (dc + 1) * S],
                    wo_sb[:, dc * HIDDEN : dc * HIDDEN + 512],
                    start=(dc == 0),
                    stop=False,
                )
                nc.tensor.matmul(
                    o2[:],
                    pt_sb[:, dc * S : (dc + 1) * S],
                    wo_sb[:, dc * HIDDEN + 512 : (dc + 1) * HIDDEN],
                    start=(dc == 0),
                    stop=False,
                )
            nc.tensor.matmul(
                o1[:], ones_col[:], bo_row[:, 0:512], start=False, stop=True
            )
            nc.tensor.matmul(
                o2[:], ones_col[:], bo_row[:, 512:768], start=False, stop=True
            )

            # ---------------- layernorm ------------------------------------
            res_sb = fin.tile([S, HIDDEN], F32, tag="res", name="res_sb")
            mu_parts = fin.tile([S, 2], F32, tag="mup", name="mu_parts")
            nc.scalar.activation(
                res_sb[:, 0:512], o1[:], AF.Copy, accum_out=mu_parts[:, 0:1]
            )
            nc.scalar.activation(
                res_sb[:, 512:768], o2[:], AF.Copy, accum_out=mu_parts[:, 1:2]
            )
            mu = fin.tile([S, 1], F32, tag="mu", name="mu")
            nc.vector.tensor_reduce(
                mu[:], mu_parts[:], axis=mybir.AxisListType.X, op=ALU.add
            )
            muv = fin.tile([S, 1], F32, tag="muv", name="muv")
            nc.vector.tensor_scalar_mul(muv[:], mu[:], 1.0 / HIDDEN)
            xc = fin.tile([S, HIDDEN], F32, tag="xc", name="xc")
            nc.vector.tensor_scalar_sub(xc[:], res_sb[:], muv[:])
            sq = fin.tile([S, HIDDEN], F32, tag="sq", name="sq")
            varsum = fin.tile([S, 1], F32, tag="vs", name="varsum")
            nc.scalar.activation(sq[:], xc[:], AF.Square, accum_out=varsum[:])
            vareps = fin.tile([S, 1], F32, tag="ve", name="vareps")
            nc.vector.tensor_scalar(
                vareps[:], varsum[:], 1.0 / HIDDEN, LN_EPS, op0=ALU.mult, op1=ALU.add
            )
            sd = fin.tile([S, 1], F32, tag="sd", name="sd")
            nc.scalar.activation(sd[:], vareps[:], AF.Sqrt)
            rstd = fin.tile([S, 1], F32, tag="rstd", name="rstd")
            nc.vector.reciprocal(rstd[:], sd[:])
            t1 = fin.tile([S, HIDDEN], F32, tag="t1", name="t1")
            nc.vector.scalar_tensor_tensor(
                t1[:], xc[:], rstd[:], gamma_rep[:], op0=ALU.mult, op1=ALU.mult
            )
            out_sb = fin.tile([S, HIDDEN], F32, tag="osb", name="out_sb")
            nc.vector.tensor_add(out_sb[:], t1[:], beta_rep[:])
            nc.sync.dma_start(out[:], out_sb[:])


_NC_CACHE = {}


def kernel(hidden_states, mask, Wq, bq, Wv, bv, Wo, bo, gamma, beta):
    if "nc" not in _NC_CACHE:
        _NC_CACHE["nc"] = build_kernel()
    nc = _NC_CACHE["nc"]
    bf = ml_dtypes.bfloat16
    f32 = np.float32

    Wq_r = np.ascontiguousarray(
        np.asarray(Wq, dtype=f32).reshape(HC, 128, HEADS).transpose(1, 0, 2)
        .reshape(128, HC * HEADS).astype(bf)
    )
    Wv_r = np.ascontiguousarray(
        np.asarray(Wv, dtype=f32).reshape(HC, 128, HIDDEN).transpose(1, 0, 2)
        .reshape(128, HC * HIDDEN).astype(bf)
    )
    Wo_r = np.ascontiguousarray(
        np.asarray(Wo, dtype=f32).reshape(HC, 128, HIDDEN).transpose(1, 0, 2)
        .reshape(128, HC * HIDDEN).astype(bf)
    )
    bv_r = np.ascontiguousarray(
        np.asarray(bv, dtype=f32).reshape(HC, 128).T
    )
    bq_b = np.asarray(bq, dtype=f32).astype(bf)
    bo_b = np.asarray(bo, dtype=f32).astype(bf)
    grep = np.ascontiguousarray(np.tile(np.asarray(gamma, dtype=f32), (S, 1)))
    brep = np.ascontiguousarray(np.tile(np.asarray(beta, dtype=f32), (S, 1)))
    ident_b = np.eye(128, dtype=f32).astype(bf)

    in_maps = [
        {
            "hs": np.ascontiguousarray(np.asarray(hidden_states[b], dtype=f32)).astype(bf),
            "mask": np.ascontiguousarray(
                np.asarray(mask[b], dtype=f32).reshape(S * T)
            ).astype(bf),
            "wq": Wq_r,
            "bq": bq_b,
            "wv": Wv_r,
            "bv": bv_r,
            "wo": Wo_r,
            "bo": bo_b,
            "grep": grep,
            "brep": brep,
            "ident": ident_b,
        }
        for b in range(N_CORES)
    ]
    res = run_bass_kernel_spmd(nc, in_maps, core_ids=list(range(N_CORES)))
    _NC_CACHE["last_results"] = res
    globals()["_LAST_RESULTS"] = res
    return np.stack([res.results[i]["out"] for i in range(N_CORES)], axis=0)
